# revision 35
# baseline (speedup 1.0000x reference)
"""Trainium2 Bass kernel for the nn_GAT problem (2-layer GAT, N=4096, H=8).

Key structural property exploited: the GAT attention score is
    score[h,i,j] = a_i[h]@x_i + a_j[h]@x_j + b[h]
The softmax is over j, and the i-dependent term (and bias) are constant
along j, so they cancel exactly: attention weights are IDENTICAL for every
query node i.  Hence

    out[i,:] = mean_h( softmax_j(s_j[h,:]) @ e1 )       (same row for all i)

and layer 2 (whose input rows are then all identical) reduces to a uniform
average, i.e. the identity on its (constant) input rows.  The whole network
collapses to one softmax-weighted global mean over nodes + two tiny linears.
This is exact in exact arithmetic for ANY input values (it is a property of
the module, not the data); numerically it matches the fp32 reference to
~6e-5 relative.

Device computation (replicated on all 8 cores, SPMD), "u-form": instead of
weighting the embedded nodes e1 = [x|1]@(W1^T;b1), weight the RAW nodes and
embed once at the end — this removes any PSUM->SBUF staging of e1:
    sj    [4096,8]  = xt_ext^T @ Wsj          (32 PE matmuls, 8-wide moving)
    p     [4096,8]  = exp(sj)                 (8 ACT ops, PSUM->SBUF)
    u_ext [8,65]    = p^T @ [x|1]             (32 accumulating PE matmuls,
                                               rhs streamed straight from DMA)
    ubar  [64]      = mean_h u[h,:]/u[h,64]   (tail)
    r1    [64]      = (W1^T;b1)^T @ [ubar;1]  ; leaky ; elu
    row   [32]      = leaky((W2^T;b2)^T @ [r1;1])
Output = row broadcast to [4096,32] (host).
"""

import sys

import numpy as np

for _p in ("/opt/trn_rl_repo",):
    if _p not in sys.path:
        sys.path.insert(0, _p)

N = 4096
CHUNK = 128
NCHUNK = N // CHUNK  # 32
KC = 65  # 64 features + ones row/col
NCORES = 8

_BUILT = {}


def _build_module(reps=1):
    import concourse.mybir as mybir
    from concourse import bacc, masks, tile

    fp32 = mybir.dt.float32

    nc = bacc.Bacc(
        "TRN2",
        target_bir_lowering=False,
        debug=False,
        num_devices=NCORES,
    )

    xt_d = nc.dram_tensor("xt_ext", [KC, N], fp32, kind="ExternalInput")
    # xr is host-packed into the exact SBUF layout [128, 32*65]
    # (xr_pack[p, n*65+w] = [x|1][n*128+p, w]) so its DMA is fully contiguous
    xr_d = nc.dram_tensor("xr_pack", [CHUNK, NCHUNK * KC], fp32, kind="ExternalInput")
    wsj_d = nc.dram_tensor("wsj", [KC, 8], fp32, kind="ExternalInput")
    w1_d = nc.dram_tensor("w1ext", [KC, 64], fp32, kind="ExternalInput")
    w2_d = nc.dram_tensor("w2ext", [KC, 32], fp32, kind="ExternalInput")
    out_d = nc.dram_tensor("row_out", [32, 1], fp32, kind="ExternalOutput")

    with tile.TileContext(nc) as tc:
        with (
            tc.tile_pool(name="const", bufs=1) as const_pool,
            tc.tile_pool(name="xt", bufs=1) as xt_pool,
            tc.tile_pool(name="work", bufs=1) as work_pool,
            tc.tile_pool(name="mm1", bufs=3, space="PSUM") as mm1_pool,
            tc.tile_pool(name="acc", bufs=1, space="PSUM") as acc_pool,
            tc.tile_pool(name="tail", bufs=2, space="PSUM") as tail_pool,
        ):
            wsj_sb = const_pool.tile([KC, 8], fp32)
            w1_sb = const_pool.tile([KC, 64], fp32)
            w2_sb = const_pool.tile([KC, 32], fp32)
            eighth_sb = const_pool.tile([8, 1], fp32)
            ub_ext = const_pool.tile([KC, 1], fp32)
            r1e_ext = const_pool.tile([KC, 1], fp32)
            identity_sb = const_pool.tile([KC, KC], fp32)
            # wsj via gpsimd (SWDGE): needed by the first matmuls, and Pool is
            # otherwise idle; w1/w2 are only needed by the tail, issued later
            nc.gpsimd.dma_start(wsj_sb[:], wsj_d[:])
            masks.make_identity(nc, identity_sb[:])
            nc.vector.memset(eighth_sb[:], 0.125)
            nc.vector.memset(ub_ext[64:65, :], 1.0)
            nc.vector.memset(r1e_ext[64:65, :], 1.0)

            for _rep in range(reps):
                # xt: 4 col-slices, xr: 4 contiguous slices; interleave issue
                # across the two HWDGE engines (SP + ACT) since descriptor
                # generation serializes per issuing engine, and keep ACT's
                # issues early so exp work isn't starved.
                # DMA issue plan (descriptor generation serializes per issuing
                # engine, ~0.8-1.6us each): slices are assigned so each
                # arrival lands just before the pipeline needs it, with ACT
                # carrying only one slice since it must be free for exps, and
                # Pool (idle otherwise) carrying the bulk.
                xt_sb = xt_pool.tile([KC, N], fp32, tag="xt")
                xr_sb = xt_pool.tile([CHUNK, NCHUNK * KC], fp32, tag="xr")

                def xt_dma(eng, k):
                    slt = slice(k * 1024, (k + 1) * 1024)
                    eng.dma_start(xt_sb[:, slt], xt_d[:, slt])

                def xr_dma(eng, k):
                    slr = slice(k * 8 * KC, (k + 1) * 8 * KC)
                    eng.dma_start(xr_sb[:, slr], xr_d[:, slr])

                # SP issue spacing ~0.8us, Pool (SWDGE) ~1.6us, ACT is free
                # for exactly one early issue before exp work begins
                xt_dma(nc.sync, 0)  # SP slot1: arr ~3.5us (needed first)
                xr_dma(nc.sync, 0)  # SP slot2: arr ~4.3
                xr_dma(nc.sync, 1)  # SP slot3: arr ~5.1
                xr_dma(nc.sync, 2)  # SP slot4: arr ~5.9
                xr_dma(nc.sync, 3)  # SP slot5: arr ~6.7
                xt_dma(nc.gpsimd, 1)  # Pool slot2: arr ~4.1
                xt_dma(nc.scalar, 2)  # ACT only slot: arr ~4.8
                xt_dma(nc.gpsimd, 3)  # Pool slot3: arr ~5.6
                if _rep == 0:
                    nc.gpsimd.dma_start(w1_sb[:], w1_d[:])
                    nc.gpsimd.dma_start(w2_sb[:], w2_d[:])

                p_sb = work_pool.tile([CHUNK, NCHUNK * 8], fp32, tag="p")
                # transposed accumulator uT[c,h] = sum_j xr[j,c] p[j,h]: with
                # xr as the stationary operand the moving width is only 8
                # columns, cutting per-matmul PE occupancy ~108ns -> ~85ns
                uT_acc = acc_pool.tile([KC, 8], fp32, tag="accT")
                # Emit all sj matmuls + exps FIRST, all weighted-sum
                # matmuls AFTER: the PE executes its stream in static order,
                # so interleaving would let a late xr slice stall queued mm2s
                # and block the independent mm1s sitting behind them.
                for g in range(4):  # 8 chunks of sj per PSUM bank
                    mm1 = mm1_pool.tile([CHUNK, 64], fp32, tag="mm1")
                    for c in range(8):
                        n = g * 8 + c
                        nc.tensor.matmul(
                            mm1[:, c * 8 : (c + 1) * 8],
                            xt_sb[:, n * CHUNK : (n + 1) * CHUNK],
                            wsj_sb[:],
                            start=True,
                            stop=True,
                        )
                    for half in range(2):  # exp 4 chunks at a time
                        b = g * 2 + half  # batch index 0..7
                        nc.scalar.activation(
                            p_sb[:, b * 32 : (b + 1) * 32],
                            mm1[:, half * 32 : (half + 1) * 32],
                            mybir.ActivationFunctionType.Exp,
                        )
                for n in range(NCHUNK):
                    nc.tensor.matmul(
                        uT_acc[:],
                        xr_sb[:, n * KC : (n + 1) * KC],
                        p_sb[:, n * 8 : (n + 1) * 8],
                        start=(n == 0),
                        stop=(n == NCHUNK - 1),
                    )

                # ---- tail ----
                uT_sb = work_pool.tile([KC, 8], fp32, tag="utsb")
                nc.vector.tensor_copy(uT_sb[:], uT_acc[:])
                u_ps = tail_pool.tile([8, KC], fp32, tag="tailps")
                nc.tensor.transpose(u_ps[:], uT_sb[:], identity_sb[:])
                inv_s = work_pool.tile([8, 1], fp32, tag="invs")
                nc.vector.reciprocal(inv_s[:], u_ps[:, 64:65])
                u_n = work_pool.tile([8, 64], fp32, tag="un")
                nc.vector.tensor_scalar_mul(u_n[:], u_ps[:, 0:64], inv_s[:])
                ubar_ps = tail_pool.tile([64, 1], fp32, tag="tailps")
                nc.tensor.matmul(
                    ubar_ps[:], u_n[:], eighth_sb[:], start=True, stop=True
                )
                nc.vector.tensor_copy(ub_ext[0:64, :], ubar_ps[:])
                r1_ps = tail_pool.tile([64, 1], fp32, tag="tailps")
                nc.tensor.matmul(r1_ps[:], w1_sb[:], ub_ext[:], start=True, stop=True)

                t02 = work_pool.tile([64, 1], fp32, tag="t02")
                lk = work_pool.tile([64, 1], fp32, tag="lk")
                mn = work_pool.tile([64, 1], fp32, tag="mn")
                ex = work_pool.tile([64, 1], fp32, tag="ex")
                rl1 = work_pool.tile([64, 1], fp32, tag="rl1")
                # leaky(x) = max(x, 0.2*x)
                nc.vector.tensor_scalar_mul(t02[:], r1_ps[:], 0.2)
                nc.vector.tensor_tensor(
                    lk[:], r1_ps[:], t02[:], op=mybir.AluOpType.max
                )
                # elu(x) = (max(x,0) - 1) + exp(min(x,0))
                nc.vector.tensor_scalar_min(mn[:], lk[:], 0.0)
                nc.scalar.activation(ex[:], mn[:], mybir.ActivationFunctionType.Exp)
                nc.vector.tensor_scalar(
                    rl1[:], lk[:], 0.0, -1.0,
                    op0=mybir.AluOpType.max, op1=mybir.AluOpType.add,
                )
                nc.vector.tensor_tensor(
                    r1e_ext[0:64, :], rl1[:], ex[:], op=mybir.AluOpType.add
                )
                r2_ps = tail_pool.tile([32, 1], fp32, tag="tailps")
                nc.tensor.matmul(r2_ps[:], w2_sb[:], r1e_ext[:], start=True, stop=True)
                out_sb = work_pool.tile([32, 1], fp32, tag="outsb")
                t2 = work_pool.tile([32, 1], fp32, tag="t2")
                nc.vector.tensor_scalar_mul(t2[:], r2_ps[:], 0.2)
                nc.vector.tensor_tensor(
                    out_sb[:], r2_ps[:], t2[:], op=mybir.AluOpType.max
                )
                nc.sync.dma_start(out_d[:], out_sb[:])

    nc.compile()
    return nc


def _get_module():
    if "nc" not in _BUILT:
        _BUILT["nc"] = _build_module(1)
    return _BUILT["nc"]


def _host_prep(x, W1, b1, a1_w, W2, b2):
    f32 = np.float32
    x = np.asarray(x, f32)
    W1 = np.asarray(W1, f32)
    b1 = np.asarray(b1, f32)
    a1_w = np.asarray(a1_w, f32)
    W2 = np.asarray(W2, f32)
    b2 = np.asarray(b2, f32)
    assert x.shape == (N, 64) and W1.shape == (64, 64) and a1_w.shape == (8, 128)
    W1T_ext = np.concatenate([W1.T, b1[None, :]], 0).astype(f32)  # [65,64]
    Wsj = (W1T_ext @ a1_w[:, 64:].T).astype(f32)  # [65,8]
    W2T_ext = np.concatenate([W2.T, b2[None, :]], 0).astype(f32)  # [65,32]
    xt_ext = np.empty((KC, N), f32)
    xt_ext[:64] = x.T
    xt_ext[64] = 1.0
    # pack [x|1] rows into the SBUF layout [128, 32*65]:
    # xr_pack[p, n*65+w] = [x|1][n*128+p, w]
    xr_pack = np.empty((CHUNK, NCHUNK, KC), f32)
    xr_pack[:, :, :64] = x.reshape(NCHUNK, CHUNK, 64).transpose(1, 0, 2)
    xr_pack[:, :, 64] = 1.0
    xr_pack = xr_pack.reshape(CHUNK, NCHUNK * KC)
    return xt_ext, xr_pack, Wsj, W1T_ext, W2T_ext


def kernel(x, W1, b1, a1_w, a1_b, W2, b2, a2_w, a2_b):
    import time

    from concourse.bass_utils import run_bass_kernel_spmd

    nc = _get_module()
    xt_ext, xr_pack, Wsj, W1T_ext, W2T_ext = _host_prep(x, W1, b1, a1_w, W2, b2)
    in_map = {
        "xt_ext": xt_ext,
        "xr_pack": xr_pack,
        "wsj": Wsj,
        "w1ext": W1T_ext,
        "w2ext": W2T_ext,
    }
    # Transient NRT_EXEC_UNIT_UNRECOVERABLE wedges happen and clear after a
    # few tens of seconds; retry with growing backoff.
    last_err = None
    for backoff in (2.0, 5.0, 15.0, 30.0, 45.0):
        try:
            res = run_bass_kernel_spmd(nc, [in_map] * NCORES, list(range(NCORES)))
            break
        except Exception as e:  # noqa: BLE001
            last_err = e
            time.sleep(backoff)
    else:
        raise last_err
    row = np.asarray(res.results[0]["row_out"], dtype=np.float32).reshape(32)
    out = np.empty((N, 32), np.float32)
    out[:] = row[None, :]
    return out


if __name__ == "__main__":
    rng = np.random.default_rng(0)
    s = lambda f: 1.0 / np.sqrt(f)
    ins = dict(
        x=rng.standard_normal((N, 64)).astype(np.float32),
        W1=(rng.standard_normal((64, 64)) * s(64)).astype(np.float32),
        b1=(rng.standard_normal(64) * s(64)).astype(np.float32),
        a1_w=(rng.standard_normal((8, 128)) * s(128)).astype(np.float32),
        a1_b=(rng.standard_normal(8) * s(128)).astype(np.float32),
        W2=(rng.standard_normal((32, 64)) * s(64)).astype(np.float32),
        b2=(rng.standard_normal(32) * s(64)).astype(np.float32),
        a2_w=(rng.standard_normal((8, 64)) * s(64)).astype(np.float32),
        a2_b=(rng.standard_normal(8) * s(64)).astype(np.float32),
    )
    out = kernel(**ins)
    print("kernel output", out.shape, out.dtype, out[0, :5])
